# revision 32
# baseline (speedup 1.0000x reference)
"""Trainium2 Bass kernel for gaussian-weighted box-feature scatter (pooling).

Math (from the reference):
    out[c,h,w] = (1/N) * sum_n box_feats[c,n] * gmaps[n,h,w]
with gmaps separable:
    gmaps[n,h,w] = exp(-(h - x1[n])^2 / (2 s_n^2)) * exp(-w^2 / (2 s_n^2))
                 = gy[n,h] * gx[n,w]

Host (tiny, O(N*C + N*(H+W))): box corner math, one bilinear sample per box
(box_feats [C,N]), and the two 1-D gaussian profiles gy [N,H], gx [N,W].

Device (heavy, O(C*H*W)): rank-N reconstruction
    out[c,h,w] = sum_n (A[c,n]*gy[n,h]) * gx[n,w],   A = box_feats/N
as per-h matmuls on the PE (lhsT = B_h[n,c] = A_T[n,c]*gy[n,h], rhs = gx),
then PSUM evacuated concurrently by the DVE *and* ACT engines with an
fp32 -> fp16 cast, and the fp16 staged rows DMA'd to HBM (host upcasts to
fp32).  fp16 halves the dominant HBM write (16.8 MB/core) and the metric
(max|err| / max|expected|, gate 2e-2) has ~4 decades of margin over the
~2^-12 quantization error.

Throughput tricks vs the naive loop:
  * B_h for 4 consecutive h are built by ONE tensor_scalar_mul on 128
    partitions (host pre-replicates A_T at partition offsets 0/32/64/96,
    and packs gy as gy4[32k+n, g] = gy[n, 4g+k]); 16 DVE ops total.
  * The 4 stationaries of a group sit at partition bases 0/32/64/96, with
    gx replicated to match.  matmul() derives tile_position from the base
    partition, so consecutive LDWEIGHTS target different PE row-strips and
    overlap with in-flight MATMULs instead of serializing.
  * PSUM is split into 4 x [128,1024] tiles (2 h-rows each); evacuation
    copies are 1024 wide to amortize the per-op bubble, and are assigned
    greedily to DVE (1192 ns) or ACT (997 ns) to balance the two queues.
  * Output DMA is chunked [4,12,24,24] h-rows per 128-channel block so the
    first descriptor hits the wire ~4 us in, keeping HBM busy end-to-end.

Sharding: H split across the 8 cores (64 rows each) - fully local.
"""

import numpy as np
from contextlib import ExitStack

from concourse import bass, tile, mybir
from concourse.tile import add_dep_helper
from concourse.bass_utils import run_bass_kernel_spmd

# Problem shapes (hardcoded per the task contract).
C, H, W = 256, 512, 512
N = 20
N_CORES = 8
HS = H // N_CORES          # 64 rows of the output per core
K3 = 3                     # h-rows per batched tensor_scalar (partition blocks
                           # at bases 0/32/64; base 96 is an illegal matmul
                           # operand base - PE quadrant 3)
G3 = (HS + K3 - 1) // K3   # 22 groups (last group only 1 valid h)
PPART = 96                 # partitions used by params / b tiles
# h-rows per output DMA, per 128-channel block.  7 chunks + the input DMA
# = 8 HWDGE descriptors exactly - every DMA instruction holds ONE sync
# wait, so none may re-use a completion sem lane.  Tapered tails keep the
# post-evac drain short; cblk0 leads with a small chunk for an early wire.
CHUNKS0 = (8, 16, 28, 12)
CHUNKS1 = (16, 36, 12)
CHUNKS = CHUNKS0 + CHUNKS1      # stage pool sizing

# params column layout: [a3 | gy3 | gx3]
A0, A1 = 0, C              # a_t replicated at partition 32k+n, k<3
GY0, GY1 = C, C + G3       # gy3[32k+n, g] = gy[n, 3g+k]
GX0, GX1 = C + G3, C + G3 + W
PF = GX1

F32 = mybir.dt.float32
F16I = mybir.dt.float16            # params dtype (halves the input DMA)
F32R = mybir.dt.float32r
F16 = mybir.dt.float16

VOXEL = (0.4, 0.4, 4.0)
LIDAR_RANGE = (-102.4, -102.4, -3.0, 102.4, 102.4, 1.0)
DOWNSAMPLE = 1

# errata-adjusted evacuation cost (ns) for a [128,1024] fp32 PSUM->SBUF copy
DVE_COPY_NS = (120 + 1024) / 0.96
ACT_COPY_NS = (172 + 1024) / 1.2
DVE_TS_NS = (58 + 256) / 0.96          # one batched B build (3 h-rows)
ACT_TS_NS = (224 + 256) / 1.2          # same, on the scalar engine

_PROG = None          # cached Bass program
LAST_RESULTS = None   # BassKernelResults of the most recent run (for test.py)


def _host_factors(pred_box_infra, infra_features):
    """Per-box scalars, bilinear-sampled box features and separable gaussian
    profiles - all tiny. Coordinate math in float32 to match the reference
    bit-for-bit where it matters (floor/clip decisions)."""
    boxes = pred_box_infra[:N].astype(np.float32)
    feat = infra_features[0]                      # [C,H,W] float32
    l_corner = boxes.min(axis=1)                  # [N,3]
    r_corner = boxes.max(axis=1)
    sx = np.float32(VOXEL[0] * DOWNSAMPLE)
    sy = np.float32(VOXEL[1] * DOWNSAMPLE)
    x1 = (l_corner[:, 0] - np.float32(LIDAR_RANGE[0])) / sx
    y1 = (l_corner[:, 1] - np.float32(LIDAR_RANGE[1])) / sy
    x2 = (r_corner[:, 0] - np.float32(LIDAR_RANGE[0])) / sx
    y2 = (r_corner[:, 1] - np.float32(LIDAR_RANGE[1])) / sy
    bev_size = (y2 - y1) * (x2 - x1)              # [N]
    cx = np.float32(0.5) * (x1 + x2)
    cy = np.float32(0.5) * (y1 + y2)

    # bilinear sample at (cy, cx), matching the reference's clip/floor
    y = np.clip(cy, 0.0, H - 1.0).astype(np.float32)
    x = np.clip(cx, 0.0, W - 1.0).astype(np.float32)
    yl = np.floor(y).astype(np.int32)
    xl = np.floor(x).astype(np.int32)
    yh = np.minimum(yl + 1, H - 1)
    xh = np.minimum(xl + 1, W - 1)
    ly = (y - yl).astype(np.float64)[None, :]     # [1,N]
    lx = (x - xl).astype(np.float64)[None, :]
    g = lambda yi, xi: feat[:, yi, xi].astype(np.float64)   # [C,N]
    box_feats = (g(yl, xl) * (1 - ly) * (1 - lx)
                 + g(yl, xh) * (1 - ly) * lx
                 + g(yh, xl) * ly * (1 - lx)
                 + g(yh, xh) * ly * lx)           # [C,N] float64

    denom = 2.0 * bev_size.astype(np.float64) ** 2          # [N]
    hh = np.arange(H, dtype=np.float64)
    ww = np.arange(W, dtype=np.float64)
    gy = np.exp(-((hh[None, :] - x1.astype(np.float64)[:, None]) ** 2) / denom[:, None])
    gx = np.exp(-(ww[None, :] ** 2) / denom[:, None])

    a_t = np.ascontiguousarray((box_feats / N).T.astype(np.float32))  # [N,C]
    return a_t, gy.astype(np.float32), gx.astype(np.float32)


def _chunk_of(h, cblk):
    """(chunk_idx, h_start, h_len) for local row h."""
    s = 0
    for ci, ln in enumerate(CHUNKS0 if cblk == 0 else CHUNKS1):
        if h < s + ln:
            return ci, s, ln
        s += ln
    raise AssertionError(h)


def _build_program():
    nc = bass.Bass("TRN2", target_bir_lowering=False, debug=False,
                   num_devices=N_CORES)
    params = nc.dram_tensor("params", [PPART, PF], F16I,
                            kind="ExternalInput").ap()
    out = nc.dram_tensor("out", [C, HS, W], F16, kind="ExternalOutput").ap()

    with ExitStack() as ctx:
        tc = ctx.enter_context(tile.TileContext(nc))
        # Every tile is allocated exactly once (bufs=1, distinct names):
        # no pool-slot recycling means no unconditional pool WAR waits, so
        # all cross-engine deps go through the subsumable dep-graph path
        # and each instruction fits its single ISA wait slot.
        const = ctx.enter_context(tc.tile_pool(name="const", bufs=1))
        bpool = ctx.enter_context(tc.tile_pool(name="bstat", bufs=1))
        ppool = ctx.enter_context(tc.tile_pool(name="psum", bufs=1, space="PSUM"))
        spools = {}
        for ln in sorted(set(CHUNKS)):
            spools[ln] = ctx.enter_context(
                tc.tile_pool(name=f"stage{ln}", bufs=1))

        def chunks_for(cblk):
            return CHUNKS0 if cblk == 0 else CHUNKS1

        # input via HWDGE (SP ring): dispatches at ~0.3us vs ~10us on the
        # SWDGE/Q7 path - everything downstream waits on this load.  fp16
        # params: half the input bytes, the b3 builds hit the DVE 4x
        # perf-mode, and the matmul streams gx straight out of p_sb.
        p_sb = const.tile([PPART, PF], F16I)
        in_dma = nc.sync.dma_start(p_sb[:], params[:])
        a3_sb = p_sb[:, A0:A1]
        # tensor_scalar's scalar operand must be fp32: tiny one-time upcast
        gy3_sb = const.tile([PPART, G3], F32)
        nc.vector.tensor_copy(gy3_sb[:], p_sb[:, GY0:GY1])
        gx3_mm = p_sb[:, GX0:GX1]

        # static fp16 stage tiles: (cblk, chunk) -> tile
        stages = {}
        for cblk in range(2):
            for ci, ln in enumerate(chunks_for(cblk)):
                stages[(cblk, ci)] = spools[ln].tile(
                    [128, ln * W], F16, name=f"stage_{cblk}_{ci}")

        # 4 static psum tiles (2 banks each = all 8 banks).  Tiles rotate
        # ct % 4 and the evac engine is fixed per cblk (ct parity), so the
        # previous reader of a reused psum tile is always the same engine:
        # program order, no sem.
        PBUFS = 4
        pstiles = [ppool.tile([128, 2 * W], F32, name=f"ps{i}")
                   for i in range(PBUFS)]

        # PE "observe" op: a standalone 2-column bf16 LDWEIGHTS (garbage
        # weights - every real matmul self-loads).  It is a real PE
        # instruction, so the sem wait it carries updates the PE's
        # observed tick and the following matmul's duplicate wait is
        # elided.  (A dummy MATMUL would need a PSUM bank; LDW does not.)

        ascratch = const.tile([1, 40], F32)
        dscratch = const.tile([128, 40], F32)
        acol = [0]
        dcol = [0]

        # A reused psum tile's copy carries {prev same-engine evac, PE
        # RAW}; a same-engine touch (never stalls - the dep is program-
        # order old) takes the first wait.  One touch covers two evacs:
        # its dep tick also dominates the next evac's older requirement.
        def dve_touch(dep_inst):
            t = nc.vector.memset(dscratch[:, dcol[0]:dcol[0] + 1], 0.0)
            dcol[0] += 1
            add_dep_helper(t.ins, dep_inst, sync=True,
                           reason="evac touch (dve)")
            return t

        def act_touch(dep_inst):
            t = nc.scalar.copy(ascratch[0:1, acol[0]:acol[0] + 1],
                               ascratch[0:1, 39:40])
            acol[0] += 1
            add_dep_helper(t.ins, dep_inst, sync=True,
                           reason="evac touch (act)")
            return t

        def pe_observe(dep_inst, why):
            # garbage fp16 weights from p_sb - every real matmul self-loads.
            # Reading p_sb keeps the only data dep on the input DMA itself.
            d = nc.tensor.ldweights(p_sb[0:2, 0:2])
            add_dep_helper(d.ins, dep_inst, sync=True, reason=why)
            return d

        eng_t = {"dve": 0.0, "act": 0.0}
        last_ev = {"dve": None, "act": None}
        ev_count = {"dve": 0, "act": 0}
        tpin = {}
        dmas = []
        last_mm = None
        btiles = {}                              # group -> (tile, op inst)

        def b_for(h):
            g = h // K3
            fresh = g not in btiles
            if fresh:
                bt = bpool.tile([PPART, C], F16, name=f"b3g{g}")
                # all on DVE: fp16 source hits the 4x perf mode (~130 ns)
                bop = nc.vector.tensor_scalar_mul(bt[:], a3_sb,
                                                  gy3_sb[:, g:g + 1])
                btiles[g] = (bt, bop.ins)
            bt, bop = btiles[g]
            pb = 32 * (h % K3)
            return bt, pb, bop, fresh

        pslot_ev = {}                            # tile counter -> evac inst
        chunk_last = {}                          # (cblk, ci) -> last evac
        pct = [0]
        first_pins = [pe_observe(in_dma.ins, "pre-cover input load")]

        for p in range(HS // 2):
            h0 = 2 * p                           # first of the h-pair
            for cblk in range(2):
                ci, cs, cln = _chunk_of(h0, cblk)
                ct = pct[0]
                pct[0] += 1
                pins = first_pins
                first_pins = []
                new_b = []
                for j in range(2):
                    bt, pb, bop, fresh = b_for(h0 + j)
                    new_b.append((bt, pb))
                    # a fresh b3 consumed by the j=0 matmul of a recycled
                    # psum tile would add a second sem wait there; a PE
                    # LDW-observe takes it first.
                    if fresh and j == 0 and ct >= PBUFS:
                        pins.append(pe_observe(bop, "pre-cover fresh b3"))
                ps = pstiles[ct % PBUFS]
                for j in range(2):
                    bt, pb = new_b[j]
                    last_mm = nc.tensor.matmul(
                        ps[:, j * W:(j + 1) * W],
                        bt[pb:pb + N, cblk * 128:(cblk + 1) * 128],
                        gx3_mm[pb:pb + N, :],
                        start=True, stop=True,
                    )
                    for d in pins:
                        add_dep_helper(last_mm.ins, d.ins, sync=False,
                                       reason="mm ordered after pre-covers")
                    pins = []
                # evacuate PSUM -> fp16 stage: cblk0 rows on the DVE,
                # cblk1 rows on the ACT.  Each chunk is single-engine, so
                # its DMA needs at most one sem wait.
                dst = stages[(cblk, ci)][:, (h0 - cs) * W:(h0 - cs + 2) * W]
                key = "dve" if cblk == 0 else "act"
                nev = ev_count[key]
                ev_count[key] += 1
                prev = last_ev[key]
                if cblk == 0:
                    if prev is not None and nev % 2 == 1:
                        t = dve_touch(prev.ins)
                        tpin[key] = t
                        eng_t["dve"] += 60
                    ev = nc.vector.tensor_copy(dst, ps[:])
                    eng_t["dve"] += DVE_COPY_NS
                else:
                    if prev is not None and nev % 2 == 1:
                        t = act_touch(prev.ins)
                        tpin[key] = t
                        eng_t["act"] += 294
                    ev = nc.scalar.copy(dst, ps[:])
                    eng_t["act"] += ACT_COPY_NS
                if tpin.get(key) is not None:
                    add_dep_helper(ev.ins, tpin[key].ins, sync=False,
                                   reason="evac ordered after touch")
                    tpin[key] = None
                last_ev[key] = ev
                pslot_ev[ct] = ev.ins
                chunk_last[(cblk, ci)] = ev
                if h0 + 2 == cs + cln:
                    # single-engine chunks: each DMA carries exactly its
                    # one data sem wait, on the otherwise-idle SP ring.
                    dma = nc.sync.dma_start(
                        out[cblk * 128:(cblk + 1) * 128, cs:cs + cln, :],
                        stages[(cblk, ci)][:].rearrange(
                            "p (h w) -> p h w", h=cln),
                    )
                    dmas.append(dma)

        # The tail drain (SP) carries one ISA wait; pre-cover every live sem
        # with single-wait SP nops so add_sem_waits elides them on the drain.
        tail_deps = [in_dma.ins, last_mm.ins] + [d.ins for d in dmas]
        for e in ("dve", "act"):
            if last_ev[e] is not None:
                tail_deps.append(last_ev[e].ins)
        for dep in tail_deps:
            tnop = nc.sync.nop(nofuse=True)
            add_dep_helper(tnop.ins, dep, sync=True,
                           reason="tail drain pre-cover")
    return nc


def _program():
    global _PROG
    if _PROG is None:
        _PROG = _build_program()
    return _PROG


def make_in_maps(pred_box_infra, infra_features):
    a_t, gy_full, gx = _host_factors(
        np.asarray(pred_box_infra, dtype=np.float32),
        np.asarray(infra_features, dtype=np.float32),
    )
    in_maps = []
    for c in range(N_CORES):
        gy_c = gy_full[:, c * HS:(c + 1) * HS]    # [N, HS]
        P = np.zeros((PPART, PF), dtype=np.float16)
        for k in range(K3):
            rows = slice(32 * k, 32 * k + N)
            P[rows, A0:A1] = a_t
            sub = gy_c[:, k::K3]                  # [N, ngroups for this k]
            P[rows, GY0:GY0 + sub.shape[1]] = sub
            P[rows, GX0:GX1] = gx
        in_maps.append({"params": P})
    return in_maps


def kernel(pred_box_infra, infra_features):
    global LAST_RESULTS
    in_maps = make_in_maps(pred_box_infra, infra_features)
    nc = _program()
    res = run_bass_kernel_spmd(nc, in_maps, core_ids=list(range(N_CORES)))
    LAST_RESULTS = res
    full = np.empty((1, C, H, W), dtype=np.float32)
    for c in range(N_CORES):
        full[0, :, c * HS:(c + 1) * HS, :] = res.results[c]["out"]
    return full


# revision 34
# speedup vs baseline: 1.0589x; 1.0589x over previous
"""Trainium2 Bass kernel for gaussian-weighted box-feature scatter (pooling).

Math (from the reference):
    out[c,h,w] = (1/N) * sum_n box_feats[c,n] * gmaps[n,h,w]
with gmaps separable:
    gmaps[n,h,w] = exp(-(h - x1[n])^2 / (2 s_n^2)) * exp(-w^2 / (2 s_n^2))
                 = gy[n,h] * gx[n,w]

Host (tiny, O(N*C + N*(H+W))): box corner math, one bilinear sample per box
(box_feats [C,N]), and the two 1-D gaussian profiles gy [N,H], gx [N,W].

Device (heavy, O(C*H*W)): rank-N reconstruction
    out[c,h,w] = sum_n (A[c,n]*gy[n,h]) * gx[n,w],   A = box_feats/N
as per-h matmuls on the PE (lhsT = B_h[n,c] = A_T[n,c]*gy[n,h], rhs = gx),
then PSUM evacuated concurrently by the DVE *and* ACT engines with an
fp32 -> fp16 cast, and the fp16 staged rows DMA'd to HBM (host upcasts to
fp32).  fp16 halves the dominant HBM write (16.8 MB/core) and the metric
(max|err| / max|expected|, gate 2e-2) has ~4 decades of margin over the
~2^-12 quantization error.

Throughput tricks vs the naive loop:
  * B_h for 4 consecutive h are built by ONE tensor_scalar_mul on 128
    partitions (host pre-replicates A_T at partition offsets 0/32/64/96,
    and packs gy as gy4[32k+n, g] = gy[n, 4g+k]); 16 DVE ops total.
  * The 4 stationaries of a group sit at partition bases 0/32/64/96, with
    gx replicated to match.  matmul() derives tile_position from the base
    partition, so consecutive LDWEIGHTS target different PE row-strips and
    overlap with in-flight MATMULs instead of serializing.
  * PSUM is split into 4 x [128,1024] tiles (2 h-rows each); evacuation
    copies are 1024 wide to amortize the per-op bubble, and are assigned
    greedily to DVE (1192 ns) or ACT (997 ns) to balance the two queues.
  * Output DMA is chunked [4,12,24,24] h-rows per 128-channel block so the
    first descriptor hits the wire ~4 us in, keeping HBM busy end-to-end.

Sharding: H split across the 8 cores (64 rows each) - fully local.
"""

import numpy as np
from contextlib import ExitStack

from concourse import bass, tile, mybir
from concourse.tile import add_dep_helper
from concourse.bass_utils import run_bass_kernel_spmd

# Problem shapes (hardcoded per the task contract).
C, H, W = 256, 512, 512
N = 20
N_CORES = 8
HS = H // N_CORES          # 64 rows of the output per core
K3 = 3                     # h-rows per batched tensor_scalar (partition blocks
                           # at bases 0/32/64; base 96 is an illegal matmul
                           # operand base - PE quadrant 3)
G3 = (HS + K3 - 1) // K3   # 22 groups (last group only 1 valid h)
PPART = 96                 # partitions used by params / b tiles
# h-rows per output DMA, per 128-channel block.  7 chunks + the input DMA
# = 8 HWDGE descriptors exactly - every DMA instruction holds ONE sync
# wait, so none may re-use a completion sem lane.  Tapered tails keep the
# post-evac drain short; cblk0 leads with a small chunk for an early wire.
CHUNKS0 = (4, 8, 12, 12, 12, 8, 8)
CHUNKS1 = (4, 8, 12, 12, 12, 8, 8)
CHUNKS = CHUNKS0 + CHUNKS1      # stage pool sizing

# params column layout: [a3 | gy3 | gx3]
A0, A1 = 0, C              # a_t replicated at partition 32k+n, k<3
GY0, GY1 = C, C + G3       # gy3[32k+n, g] = gy[n, 3g+k]
GX0, GX1 = C + G3, C + G3 + W
PF = GX1

F32 = mybir.dt.float32
F16I = mybir.dt.float16            # params dtype (halves the input DMA)
F32R = mybir.dt.float32r
F16 = mybir.dt.float16

VOXEL = (0.4, 0.4, 4.0)
LIDAR_RANGE = (-102.4, -102.4, -3.0, 102.4, 102.4, 1.0)
DOWNSAMPLE = 1

# errata-adjusted evacuation cost (ns) for a [128,1024] fp32 PSUM->SBUF copy
DVE_COPY_NS = (120 + 1024) / 0.96
ACT_COPY_NS = (172 + 1024) / 1.2
DVE_TS_NS = 273.0                      # one batched B build (3 h-rows, 2x)
ACT_TS_NS = 400.0                      # same, on the scalar engine

_PROG = None          # cached Bass program
LAST_RESULTS = None   # BassKernelResults of the most recent run (for test.py)


def _host_factors(pred_box_infra, infra_features):
    """Per-box scalars, bilinear-sampled box features and separable gaussian
    profiles - all tiny. Coordinate math in float32 to match the reference
    bit-for-bit where it matters (floor/clip decisions)."""
    boxes = pred_box_infra[:N].astype(np.float32)
    feat = infra_features[0]                      # [C,H,W] float32
    l_corner = boxes.min(axis=1)                  # [N,3]
    r_corner = boxes.max(axis=1)
    sx = np.float32(VOXEL[0] * DOWNSAMPLE)
    sy = np.float32(VOXEL[1] * DOWNSAMPLE)
    x1 = (l_corner[:, 0] - np.float32(LIDAR_RANGE[0])) / sx
    y1 = (l_corner[:, 1] - np.float32(LIDAR_RANGE[1])) / sy
    x2 = (r_corner[:, 0] - np.float32(LIDAR_RANGE[0])) / sx
    y2 = (r_corner[:, 1] - np.float32(LIDAR_RANGE[1])) / sy
    bev_size = (y2 - y1) * (x2 - x1)              # [N]
    cx = np.float32(0.5) * (x1 + x2)
    cy = np.float32(0.5) * (y1 + y2)

    # bilinear sample at (cy, cx), matching the reference's clip/floor
    y = np.clip(cy, 0.0, H - 1.0).astype(np.float32)
    x = np.clip(cx, 0.0, W - 1.0).astype(np.float32)
    yl = np.floor(y).astype(np.int32)
    xl = np.floor(x).astype(np.int32)
    yh = np.minimum(yl + 1, H - 1)
    xh = np.minimum(xl + 1, W - 1)
    ly = (y - yl).astype(np.float64)[None, :]     # [1,N]
    lx = (x - xl).astype(np.float64)[None, :]
    g = lambda yi, xi: feat[:, yi, xi].astype(np.float64)   # [C,N]
    box_feats = (g(yl, xl) * (1 - ly) * (1 - lx)
                 + g(yl, xh) * (1 - ly) * lx
                 + g(yh, xl) * ly * (1 - lx)
                 + g(yh, xh) * ly * lx)           # [C,N] float64

    denom = 2.0 * bev_size.astype(np.float64) ** 2          # [N]
    hh = np.arange(H, dtype=np.float64)
    ww = np.arange(W, dtype=np.float64)
    gy = np.exp(-((hh[None, :] - x1.astype(np.float64)[:, None]) ** 2) / denom[:, None])
    gx = np.exp(-(ww[None, :] ** 2) / denom[:, None])

    a_t = np.ascontiguousarray((box_feats / N).T.astype(np.float32))  # [N,C]
    return a_t, gy.astype(np.float32), gx.astype(np.float32)


def _chunk_of(h, cblk):
    """(chunk_idx, h_start, h_len) for local row h."""
    s = 0
    for ci, ln in enumerate(CHUNKS0 if cblk == 0 else CHUNKS1):
        if h < s + ln:
            return ci, s, ln
        s += ln
    raise AssertionError(h)


def _build_program():
    nc = bass.Bass("TRN2", target_bir_lowering=False, debug=False,
                   num_devices=N_CORES)
    params = nc.dram_tensor("params", [PPART, PF], F16I,
                            kind="ExternalInput").ap()
    out = nc.dram_tensor("out", [C, HS, W], F16, kind="ExternalOutput").ap()

    with ExitStack() as ctx:
        tc = ctx.enter_context(tile.TileContext(nc))
        # Every tile is allocated exactly once (bufs=1, distinct names):
        # no pool-slot recycling means no unconditional pool WAR waits, so
        # all cross-engine deps go through the subsumable dep-graph path
        # and each instruction fits its single ISA wait slot.
        const = ctx.enter_context(tc.tile_pool(name="const", bufs=1))
        bpool = ctx.enter_context(tc.tile_pool(name="bstat", bufs=1))
        ppool = ctx.enter_context(tc.tile_pool(name="psum", bufs=1, space="PSUM"))
        spools = {}
        for ln in sorted(set(CHUNKS)):
            spools[ln] = ctx.enter_context(
                tc.tile_pool(name=f"stage{ln}", bufs=1))

        def chunks_for(cblk):
            return CHUNKS0 if cblk == 0 else CHUNKS1

        # input via HWDGE (SP ring): dispatches at ~0.3us vs ~10us on the
        # SWDGE/Q7 path - everything downstream waits on this load.  fp16
        # params: half the input bytes, the b3 builds hit the DVE 4x
        # perf-mode, and the matmul streams gx straight out of p_sb.
        p_sb = const.tile([PPART, PF], F16I)
        in_dma = nc.sync.dma_start(p_sb[:], params[:])
        a3_sb = p_sb[:, A0:A1]
        # tensor_scalar's scalar operand must be fp32: tiny one-time upcast
        gy3_sb = const.tile([PPART, G3], F32)
        nc.vector.tensor_copy(gy3_sb[:], p_sb[:, GY0:GY1])
        gx3_mm = p_sb[:, GX0:GX1]

        # static fp16 stage tiles: (cblk, chunk) -> tile
        stages = {}
        for cblk in range(2):
            for ci, ln in enumerate(chunks_for(cblk)):
                stages[(cblk, ci)] = spools[ln].tile(
                    [128, ln * W], F16, name=f"stage_{cblk}_{ci}")

        # 4 static psum tiles (2 banks each = all 8 banks).  Tiles rotate
        # ct % 4 and the evac engine is fixed per cblk (ct parity), so the
        # previous reader of a reused psum tile is always the same engine:
        # program order, no sem.
        PBUFS = 4
        pstiles = [ppool.tile([128, 2 * W], F32, name=f"ps{i}")
                   for i in range(PBUFS)]

        # PE "observe" op: a standalone 2-column bf16 LDWEIGHTS (garbage
        # weights - every real matmul self-loads).  It is a real PE
        # instruction, so the sem wait it carries updates the PE's
        # observed tick and the following matmul's duplicate wait is
        # elided.  (A dummy MATMUL would need a PSUM bank; LDW does not.)

        ascratch = const.tile([1, 40], F32)
        dscratch = const.tile([128, 40], F32)
        acol = [0]
        dcol = [0]

        # A reused psum tile's copy carries {prev same-engine evac, PE
        # RAW}; a same-engine touch (never stalls - the dep is program-
        # order old) takes the first wait.  One touch covers two evacs:
        # its dep tick also dominates the next evac's older requirement.
        def dve_touch(dep_inst):
            t = nc.vector.memset(dscratch[:, dcol[0]:dcol[0] + 1], 0.0)
            dcol[0] += 1
            add_dep_helper(t.ins, dep_inst, sync=True,
                           reason="evac touch (dve)")
            return t

        def act_touch(dep_inst):
            t = nc.scalar.copy(ascratch[0:1, acol[0]:acol[0] + 1],
                               ascratch[0:1, 39:40])
            acol[0] += 1
            add_dep_helper(t.ins, dep_inst, sync=True,
                           reason="evac touch (act)")
            return t

        def pe_observe(dep_inst, why):
            # garbage fp16 weights from p_sb - every real matmul self-loads.
            # Reading p_sb keeps the only data dep on the input DMA itself.
            d = nc.tensor.ldweights(p_sb[0:2, 0:2])
            add_dep_helper(d.ins, dep_inst, sync=True, reason=why)
            return d

        eng_t = {"dve": 0.0, "act": 0.0}
        last_ev = {"dve": None, "act": None}
        ev_count = {"dve": 0, "act": 0}
        tpin = {}
        dmas = []
        last_mm = None
        btiles = {}                              # group -> (tile, op inst)

        def b_for(h):
            g = h // K3
            fresh = g not in btiles
            if fresh:
                bt = bpool.tile([PPART, C], F16, name=f"b3g{g}")
                # all on DVE (fp16 2x perf mode, ~273 ns); an ACT version
                # trips the walrus wait budget (pointer-scale needs a slot)
                bop = nc.vector.tensor_scalar_mul(bt[:], a3_sb,
                                                  gy3_sb[:, g:g + 1])
                eng_t["dve"] += DVE_TS_NS
                btiles[g] = (bt, bop.ins)
            bt, bop = btiles[g]
            pb = 32 * (h % K3)
            return bt, pb, bop, fresh

        pslot_ev = {}                            # tile counter -> evac inst
        chunk_last = {}                          # (cblk, ci) -> last evac
        pct = [0]
        first_pins = [pe_observe(in_dma.ins, "pre-cover input load")]

        for p in range(HS // 2):
            h0 = 2 * p                           # first of the h-pair
            for cblk in range(2):
                ci, cs, cln = _chunk_of(h0, cblk)
                ct = pct[0]
                pct[0] += 1
                pins = first_pins
                first_pins = []
                new_b = []
                for j in range(2):
                    bt, pb, bop, fresh = b_for(h0 + j)
                    new_b.append((bt, pb))
                    # a fresh b3 consumed by the j=0 matmul of a recycled
                    # psum tile would add a second sem wait there; a PE
                    # LDW-observe takes it first.
                    if fresh and j == 0 and ct >= PBUFS:
                        pins.append(pe_observe(bop, "pre-cover fresh b3"))
                ps = pstiles[ct % PBUFS]
                for j in range(2):
                    bt, pb = new_b[j]
                    last_mm = nc.tensor.matmul(
                        ps[:, j * W:(j + 1) * W],
                        bt[pb:pb + N, cblk * 128:(cblk + 1) * 128],
                        gx3_mm[pb:pb + N, :],
                        start=True, stop=True,
                    )
                    for d in pins:
                        add_dep_helper(last_mm.ins, d.ins, sync=False,
                                       reason="mm ordered after pre-covers")
                    pins = []
                # evacuate PSUM -> fp16 stage: cblk0 rows on the DVE,
                # cblk1 rows on the ACT.  Each chunk is single-engine, so
                # its DMA needs at most one sem wait.
                dst = stages[(cblk, ci)][:, (h0 - cs) * W:(h0 - cs + 2) * W]
                key = "dve" if cblk == 0 else "act"
                nev = ev_count[key]
                ev_count[key] += 1
                prev = last_ev[key]
                if cblk == 0:
                    if prev is not None and nev % 2 == 1:
                        t = dve_touch(prev.ins)
                        tpin[key] = t
                        eng_t["dve"] += 60
                    ev = nc.vector.tensor_copy(dst, ps[:])
                    eng_t["dve"] += DVE_COPY_NS
                else:
                    if prev is not None and nev % 2 == 1:
                        t = act_touch(prev.ins)
                        tpin[key] = t
                        eng_t["act"] += 294
                    ev = nc.scalar.copy(dst, ps[:])
                    eng_t["act"] += ACT_COPY_NS
                if tpin.get(key) is not None:
                    add_dep_helper(ev.ins, tpin[key].ins, sync=False,
                                   reason="evac ordered after touch")
                    tpin[key] = None
                last_ev[key] = ev
                pslot_ev[ct] = ev.ins
                chunk_last[(cblk, ci)] = ev
                if h0 + 2 == cs + cln:
                    # single-engine chunks: each DMA carries exactly one
                    # data sem wait.  Only 8 HWDGE completion lanes exist
                    # (input + 7 chunks); the other 7 chunks ride SWDGE,
                    # which has its own completion path - its ~1-2us Q7
                    # latency is irrelevant mid-stream and its throughput
                    # matches (same SDMA engines underneath).
                    hw = (cblk == 0) == (ci % 2 == 0)
                    eng = nc.sync if hw else nc.gpsimd
                    dma = eng.dma_start(
                        out[cblk * 128:(cblk + 1) * 128, cs:cs + cln, :],
                        stages[(cblk, ci)][:].rearrange(
                            "p (h w) -> p h w", h=cln),
                    )
                    dmas.append(dma)

        # The tail drain (SP) carries one ISA wait; pre-cover every live sem
        # with single-wait SP nops so add_sem_waits elides them on the drain.
        tail_deps = [in_dma.ins, last_mm.ins] + [d.ins for d in dmas]
        for e in ("dve", "act"):
            if last_ev[e] is not None:
                tail_deps.append(last_ev[e].ins)
        for dep in tail_deps:
            tnop = nc.sync.nop(nofuse=True)
            add_dep_helper(tnop.ins, dep, sync=True,
                           reason="tail drain pre-cover")
    return nc


def _program():
    global _PROG
    if _PROG is None:
        _PROG = _build_program()
    return _PROG


def make_in_maps(pred_box_infra, infra_features):
    a_t, gy_full, gx = _host_factors(
        np.asarray(pred_box_infra, dtype=np.float32),
        np.asarray(infra_features, dtype=np.float32),
    )
    in_maps = []
    for c in range(N_CORES):
        gy_c = gy_full[:, c * HS:(c + 1) * HS]    # [N, HS]
        P = np.zeros((PPART, PF), dtype=np.float16)
        for k in range(K3):
            rows = slice(32 * k, 32 * k + N)
            P[rows, A0:A1] = a_t
            sub = gy_c[:, k::K3]                  # [N, ngroups for this k]
            P[rows, GY0:GY0 + sub.shape[1]] = sub
            P[rows, GX0:GX1] = gx
        in_maps.append({"params": P})
    return in_maps


def kernel(pred_box_infra, infra_features):
    global LAST_RESULTS
    in_maps = make_in_maps(pred_box_infra, infra_features)
    nc = _program()
    res = run_bass_kernel_spmd(nc, in_maps, core_ids=list(range(N_CORES)))
    LAST_RESULTS = res
    full = np.empty((1, C, H, W), dtype=np.float32)
    for c in range(N_CORES):
        full[0, :, c * HS:(c + 1) * HS, :] = res.results[c]["out"]
    return full
